# revision 3
# baseline (speedup 1.0000x reference)
"""Trainium2 Bass kernel for nn_BitfieldLinear (vq_codebook).

Reference computation:
    idx   = codes & 0xFF            (basis row, 256 entries)
    r_q   = (codes >> 8) & 0xFFF
    sign  = bit20 ? -1 : +1
    scale = sign * tanh(r_q / 4095)
    W     = scale[:, None] * basis[idx]        # [8192, 4096]
    y     = x @ W.T                            # [128, 8192]

Key factorization (never materialize the 128MB W):
    Z = x @ basis.T                            # [128, 256]  tiny matmul
    y[b, j] = scale[j] * Z[b, idx[j]]          # column gather + scale

The gather is a matmul with a one-hot matrix built in [basis, code]
layout directly (is_eq against a partition iota); the per-code scale is
applied on the PSUM->SBUF copy of y (tensor_tensor mult with a
partition-broadcast scale row).

Sharding: out_features column-parallel across 8 cores (1024 codes per
core); x and basis replicated (8-core collectives are ~68us on this
harness -- far slower than recomputing Z per core).

v3 layout (from v2 trace analysis):
  - exactly 8 input DMAs on the two HWDGE queues (sync+scalar), c128
    first: with <=8 in-flight HWDGE DMAs each completion gets its own
    DMAHW sem lane, so consumers wait on precise completion events (v2
    had 13 DMAs over 8 lanes; merged waits stalled the first matmul and
    the decode ~3.5us past their data).
  - PE HAM warmup: ~10 dummy matmuls on a memset scratch tile while the
    stream runs, so Z matmuls run at 2.4GHz (109ns) not 1.2GHz (213ns).
  - no G^T transposes: decode -> one [128,16] PE transpose ->
    gpsimd.partition_broadcast (idle engine) -> G one-hot built on DVE
    in [basis, code] layout; scale folded into the output copy.
  - no gpsimd DMAs, no ACT-engine ops (dodges the 1.3us ACT table load).
QUANT="fp8" streams x/basis as fp8e3m4 (pre-scaled by 2/64, compensated
in the tanh coeffs): 1.5MB/core stream at ~1.4% rel err.
"""

import os
import sys

for _p in ("/opt/trn_rl_repo", "/opt/pypackages"):
    if _p not in sys.path:
        sys.path.insert(0, _p)

import numpy as np

import concourse.bacc as bacc
import concourse.mybir as mybir
import concourse.tile as tile
from concourse.alu_op_type import AluOpType
from concourse.bass_utils import run_bass_kernel_spmd

N_CORES = 8
BATCH = 128
IN_F = 4096
OUT_F = 8192
BASIS = 256
OPC = OUT_F // N_CORES      # 1024 output columns per core
NK = IN_F // 128            # 32 K-tiles
NT = OPC // 128             # 8 code-tiles per core
R_LEVELS = 4095.0

F32 = mybir.dt.float32
BF16 = mybir.dt.bfloat16
FP16 = mybir.dt.float16
FP8 = mybir.dt.float8e3
I32 = mybir.dt.int32

QUANT = os.environ.get("BITF_QUANT", "fp8")
BCAST = os.environ.get("BITF_BCAST", "gp")   # 'gp' gpsimd | 'pe' matmul
N_WARM = int(os.environ.get("BITF_WARM", "10"))
X_DT = FP8 if QUANT == "fp8" else FP16
B_DT = FP16 if QUANT == "fp16" else FP8
X_SCALE = 2.0 if QUANT == "fp8" else 1.0     # keep fp8e3m4 out of denormals
B_SCALE = 64.0 if QUANT in ("fp8", "fp8b") else 1.0
_COMP = 1.0 / (X_SCALE * B_SCALE)            # folded into tanh coeffs

# tanh(r) ~= r*(c0 + c1 u + c2 u^2 + c3 u^3), u=r^2, r in [0,1]
# (max rel err 8e-5, negligible vs the fp8 input error); coeffs carry
# the fp8 pre-scale compensation
TANH_C = [c * _COMP for c in (
    9.9991860534e-01, -3.3065536868e-01, 1.1890093882e-01,
    -2.6632289374e-02)]

# input chunking: [k-tile start, k-tile end) per basis chunk
B_CHUNKS = [(0, 8), (8, 16), (16, 24), (24, 28), (28, 32)]
X_CHUNKS = [(0, 16), (16, 32)]


def build_nc():
    nc = bacc.Bacc(
        "TRN2",
        target_bir_lowering=False,
        debug=False,
        num_devices=N_CORES,
    )

    c128_d = nc.dram_tensor("c128", [128, NT], I32, kind="ExternalInput")
    xd = [
        nc.dram_tensor(f"xc{i}", [128, (e - s) * 128], X_DT,
                       kind="ExternalInput")
        for i, (s, e) in enumerate(X_CHUNKS)
    ]
    bd = [
        nc.dram_tensor(f"bc{i}", [128, (e - s) * 256], B_DT,
                       kind="ExternalInput")
        for i, (s, e) in enumerate(B_CHUNKS)
    ]
    out_d = nc.dram_tensor("out", [128, OPC], FP16, kind="ExternalOutput")

    with tile.TileContext(nc) as tc:
        with (
            tc.tile_pool(name="pool", bufs=1) as pool,
            tc.tile_pool(name="zps", bufs=1, space="PSUM") as zps,
            tc.tile_pool(name="tps", bufs=2, space="PSUM") as tps,
            tc.tile_pool(name="yps", bufs=2, space="PSUM") as yps,
        ):
            # ---- PE warmup: HAM un-throttles after ~3.4us of sustained
            # activity; burn it on a memset scratch while the stream runs
            scr = pool.tile([128, 256], BF16)
            nc.gpsimd.memset(scr[:], 0.0)
            for w in range(N_WARM):
                wp = tps.tile([128, 256], F32, tag="warm", name=f"warm{w}")
                nc.tensor.matmul(
                    wp[:], lhsT=scr[:, 0:128], rhs=scr[:],
                    start=True, stop=True,
                )

            # ---- input DMAs: c128 first (gates decode); 8 total on the
            # two HWDGE queues = one DMAHW sem lane each (precise waits)
            c128 = pool.tile([128, NT], I32)
            nc.sync.dma_start(out=c128[:], in_=c128_d[:])

            x_sb = pool.tile([128, IN_F], X_DT)
            b_sb = pool.tile([128, 2 * IN_F], B_DT)
            s, e = X_CHUNKS[0]
            nc.scalar.dma_start(out=x_sb[:, s * 128:e * 128], in_=xd[0][:])
            s, e = B_CHUNKS[0]
            nc.sync.dma_start(out=b_sb[:, s * 256:e * 256], in_=bd[0][:])
            s, e = B_CHUNKS[1]
            nc.scalar.dma_start(out=b_sb[:, s * 256:e * 256], in_=bd[1][:])
            s, e = X_CHUNKS[1]
            nc.sync.dma_start(out=x_sb[:, s * 128:e * 128], in_=xd[1][:])
            s, e = B_CHUNKS[2]
            nc.scalar.dma_start(out=b_sb[:, s * 256:e * 256], in_=bd[2][:])
            s, e = B_CHUNKS[3]
            nc.sync.dma_start(out=b_sb[:, s * 256:e * 256], in_=bd[3][:])
            s, e = B_CHUNKS[4]
            nc.scalar.dma_start(out=b_sb[:, s * 256:e * 256], in_=bd[4][:])

            # ---- constants: iota row/partition, bf16 identity
            iota_row_i = pool.tile([128, 128], I32)
            nc.gpsimd.iota(out=iota_row_i[:], pattern=[[1, 128]], base=0,
                           channel_multiplier=0)
            iota_part_i = pool.tile([128, 1], I32)
            nc.gpsimd.iota(out=iota_part_i[:], pattern=[[1, 1]], base=0,
                           channel_multiplier=1)
            iota_part2_i = pool.tile([128, 1], I32)
            nc.gpsimd.iota(out=iota_part2_i[:], pattern=[[1, 1]], base=128,
                           channel_multiplier=1)

            iota_f = pool.tile([128, 128], F32)
            nc.vector.tensor_scalar_mul(out=iota_f[:], in0=iota_row_i[:],
                                        scalar1=1.0)
            iota_part_f = pool.tile([128, 1], F32)
            nc.vector.tensor_scalar_mul(out=iota_part_f[:],
                                        in0=iota_part_i[:], scalar1=1.0)
            iota_part2_f = pool.tile([128, 1], F32)
            nc.vector.tensor_scalar_mul(out=iota_part2_f[:],
                                        in0=iota_part2_i[:], scalar1=1.0)
            identb = pool.tile([128, 128], BF16)
            nc.vector.tensor_scalar(
                out=identb[:], in0=iota_f[:],
                scalar1=iota_part_f[:, 0:1], scalar2=None,
                op0=AluOpType.is_equal,
            )

            # ---- decode codes -> dec[:,0:8]=idx (f32), dec[:,8:16]=scl
            dec = pool.tile([128, 16], F32)
            idx_i = pool.tile([128, NT], I32, name="idx_i")
            nc.vector.tensor_scalar(
                out=idx_i[:], in0=c128[:],
                scalar1=255, scalar2=None, op0=AluOpType.bitwise_and,
            )
            nc.vector.tensor_scalar_mul(out=dec[:, 0:8], in0=idx_i[:],
                                        scalar1=1.0)
            rq_i = pool.tile([128, NT], I32, name="rq_i")
            nc.vector.tensor_scalar(
                out=rq_i[:], in0=c128[:],
                scalar1=8, scalar2=4095,
                op0=AluOpType.logical_shift_right,
                op1=AluOpType.bitwise_and,
            )
            r = pool.tile([128, NT], F32, name="r")
            nc.vector.tensor_scalar_mul(out=r[:], in0=rq_i[:],
                                        scalar1=1.0 / R_LEVELS)
            u = pool.tile([128, NT], F32, name="u")
            nc.vector.tensor_tensor(out=u[:], in0=r[:], in1=r[:],
                                    op=AluOpType.mult)
            p = pool.tile([128, NT], F32, name="p")
            nc.vector.tensor_scalar(
                out=p[:], in0=u[:], scalar1=TANH_C[3], scalar2=TANH_C[2],
                op0=AluOpType.mult, op1=AluOpType.add,
            )
            for ci in (1, 0):
                nc.vector.tensor_tensor(out=p[:], in0=p[:], in1=u[:],
                                        op=AluOpType.mult)
                nc.vector.tensor_scalar(
                    out=p[:], in0=p[:], scalar1=TANH_C[ci], scalar2=None,
                    op0=AluOpType.add,
                )
            th = pool.tile([128, NT], F32, name="th")
            nc.vector.tensor_tensor(out=th[:], in0=p[:], in1=r[:],
                                    op=AluOpType.mult)
            sg_i = pool.tile([128, NT], I32, name="sg_i")
            nc.vector.tensor_scalar(
                out=sg_i[:], in0=c128[:],
                scalar1=20, scalar2=1,
                op0=AluOpType.logical_shift_right,
                op1=AluOpType.bitwise_and,
            )
            sgn = pool.tile([128, NT], F32, name="sgn")
            nc.vector.tensor_scalar(
                out=sgn[:], in0=sg_i[:],
                scalar1=-2.0, scalar2=1.0,
                op0=AluOpType.mult, op1=AluOpType.add,
            )
            nc.vector.tensor_tensor(out=dec[:, 8:16], in0=th[:], in1=sgn[:],
                                    op=AluOpType.mult)

            # ---- dec -> bf16 -> PE transpose -> [16, 128] in SBUF
            decb = pool.tile([128, 16], BF16)
            nc.vector.tensor_copy(out=decb[:], in_=dec[:])
            decT_ps = tps.tile([16, 128], BF16, tag="warm", name="decT_ps")
            nc.tensor.transpose(out=decT_ps[:], in_=decb[:],
                                identity=identb[:])
            decT_sb = pool.tile([16, 128], BF16)
            nc.vector.tensor_copy(out=decT_sb[:], in_=decT_ps[:])

            # partition_broadcast requires its source on partition 0:
            # flatten the 16 rows into one [1, 2048] row with a tiny
            # SBUF->SBUF SWDGE DMA (16 contiguous 256B descriptors)
            rows = pool.tile([1, 16 * 128], BF16)
            nc.gpsimd.dma_start(out=rows[:], in_=decT_sb[:])

            # ---- broadcast idx/scl rows across partitions
            idxb = pool.tile([128, OPC], BF16)
            sclb = pool.tile([128, OPC], BF16)
            for t in range(NT):
                nc.gpsimd.partition_broadcast(
                    idxb[:, t * 128:(t + 1) * 128],
                    rows[0:1, t * 128:(t + 1) * 128],
                )
            for t in range(NT):
                nc.gpsimd.partition_broadcast(
                    sclb[:, t * 128:(t + 1) * 128],
                    rows[0:1, (8 + t) * 128:(9 + t) * 128],
                )

            # ---- G one-hot halves in [basis, code] layout (bf16)
            g0 = pool.tile([128, OPC], BF16)
            g1 = pool.tile([128, OPC], BF16)
            for t in range(NT):
                nc.vector.tensor_scalar(
                    out=g0[:, t * 128:(t + 1) * 128],
                    in0=idxb[:, t * 128:(t + 1) * 128],
                    scalar1=iota_part_f[:, 0:1], scalar2=None,
                    op0=AluOpType.is_equal,
                )
                nc.vector.tensor_scalar(
                    out=g1[:, t * 128:(t + 1) * 128],
                    in0=idxb[:, t * 128:(t + 1) * 128],
                    scalar1=iota_part2_f[:, 0:1], scalar2=None,
                    op0=AluOpType.is_equal,
                )

            # ---- Z accumulation [128b, 256] over 32 K-tiles
            z_ps = zps.tile([128, BASIS], F32, tag="z")
            for k in range(NK):
                nc.tensor.matmul(
                    z_ps[:],
                    lhsT=x_sb[:, k * 128:(k + 1) * 128],
                    rhs=b_sb[:, k * 256:(k + 1) * 256],
                    start=(k == 0), stop=(k == NK - 1),
                )

            # ---- Z -> bf16, PE-transpose into Z^T halves
            z_sb = pool.tile([128, BASIS], BF16)
            nc.vector.tensor_copy(out=z_sb[:, 0:128], in_=z_ps[:, 0:128])
            nc.vector.tensor_copy(out=z_sb[:, 128:256], in_=z_ps[:, 128:256])
            zt = []
            for h in range(2):
                ztp = tps.tile([128, 128], BF16, tag="warm", name=f"ztp{h}")
                nc.tensor.transpose(
                    out=ztp[:], in_=z_sb[:, h * 128:(h + 1) * 128],
                    identity=identb[:],
                )
                ztt = pool.tile([128, 128], BF16, tag=f"zt{h}", name=f"zt{h}")
                nc.vector.tensor_copy(out=ztt[:], in_=ztp[:])
                zt.append(ztt)

            # ---- y = Z^T.T @ G per 512-col PSUM bank; scale on the
            # PSUM->SBUF copy (tensor_tensor mult with sclb); store each
            # bank as soon as it closes, on separate HWDGE queues
            for nch in range(2):
                y_ps = yps.tile([128, 512], F32, tag=f"y{nch}",
                                name=f"y_ps{nch}")
                nc.tensor.matmul(
                    y_ps[:], lhsT=zt[0][:],
                    rhs=g0[:, nch * 512:(nch + 1) * 512],
                    start=True, stop=False,
                )
                nc.tensor.matmul(
                    y_ps[:], lhsT=zt[1][:],
                    rhs=g1[:, nch * 512:(nch + 1) * 512],
                    start=False, stop=True,
                )
                y_sb = pool.tile([128, 512], FP16, tag=f"ysb{nch}",
                                 name=f"y_sb{nch}")
                nc.vector.tensor_tensor(
                    out=y_sb[:], in0=y_ps[:],
                    in1=sclb[:, nch * 512:(nch + 1) * 512],
                    op=AluOpType.mult,
                )
                if nch == 0:
                    nc.scalar.dma_start(out=out_d[:, 0:512], in_=y_sb[:])
                else:
                    nc.sync.dma_start(out=out_d[:, 512:1024], in_=y_sb[:])

    nc.compile()
    return nc


_NC = None


def _get_nc():
    global _NC
    if _NC is None:
        _NC = build_nc()
    return _NC


def make_in_maps(x, codes, basis):
    import ml_dtypes

    x = np.ascontiguousarray(x, dtype=np.float32)
    basis = np.ascontiguousarray(basis, dtype=np.float32)
    codes = np.ascontiguousarray(codes, dtype=np.int32)
    f8 = ml_dtypes.float8_e3m4
    x_np_dt = f8 if QUANT == "fp8" else np.float16
    b_np_dt = np.float16 if QUANT == "fp16" else f8

    # xt[p, k*128 + m] = x[m, k*128 + p]
    xt = np.ascontiguousarray(
        (x * X_SCALE).reshape(BATCH, NK, 128).transpose(2, 1, 0)
        .reshape(128, IN_F)
    ).astype(x_np_dt)
    # bt[p, k*256 + o] = basis[o, k*128 + p]
    bt = np.ascontiguousarray(
        (basis * B_SCALE).reshape(BASIS, NK, 128).transpose(2, 1, 0)
        .reshape(128, 2 * IN_F)
    ).astype(b_np_dt)

    shared = {}
    for i, (s, e) in enumerate(X_CHUNKS):
        shared[f"xc{i}"] = np.ascontiguousarray(xt[:, s * 128:e * 128])
    for i, (s, e) in enumerate(B_CHUNKS):
        shared[f"bc{i}"] = np.ascontiguousarray(bt[:, s * 256:e * 256])

    in_maps = []
    for c in range(N_CORES):
        sh = codes[c * OPC:(c + 1) * OPC]
        # wrap-128 layout: c128[p, t] = codes[t*128 + p]
        c128 = np.ascontiguousarray(sh.reshape(NT, 128).T)
        in_maps.append({**shared, "c128": c128})
    return in_maps


def assemble_output(results):
    return np.concatenate(
        [results[c]["out"].astype(np.float32) for c in range(N_CORES)], axis=1
    )


def kernel(x, codes, basis):
    nc = _get_nc()
    in_maps = make_in_maps(x, codes, basis)
    res = run_bass_kernel_spmd(nc, in_maps, list(range(N_CORES)))
    return assemble_output(res.results)


if __name__ == "__main__":
    rng = np.random.default_rng(0)
    x = rng.standard_normal((BATCH, IN_F), dtype=np.float32)
    basis = (rng.standard_normal((BASIS, IN_F)) * 0.02).astype(np.float32)
    codes = rng.integers(0, 1 << 22, size=(OUT_F,), dtype=np.int32)
    y = kernel(x, codes, basis)

    idx = codes & 255
    r = ((codes >> 8) & 4095).astype(np.float32) / R_LEVELS
    sign = np.where(((codes >> 20) & 1) == 1, -1.0, 1.0).astype(np.float32)
    scale = sign * np.tanh(r)
    W = scale[:, None] * basis[idx]
    y_ref = x @ W.T
    err = np.linalg.norm(y - y_ref) / np.linalg.norm(y_ref)
    print("rel err:", err)
